# revision 12
# baseline (speedup 1.0000x reference)
"""RIENet loss kernel (keypoint/KNN MSE + global-align Huber-min loss) on 8 trn2 cores.

Sharding: core ci -> (b = ci // 4, n-chunk j = ci % 4).  Each core holds the full
tgt[b] (M=8192 points) and a 2048-column chunk of src_transformed[b] (N axis).
  loss_1 (min over M per src point): per-core partial over the partition axis,
          finished on host (min over 128 partitions of the DMA'd acc tile).
  loss_2 (min over N per tgt point): per-core partial min over its chunk;
          host min-reduces the 4 chunks per batch element.

v5: all operand prep happens on the HOST (bf16 2-way splits, norms, row
layouts) so the device runs only the steady-state loop:
  PE     : Q[m-tile, :] = -2 t.s + ||s||^2 via K=11 bf16 matmuls (4 banks)
  ScalarE: Qb = bf16(Q + ||t_m||^2)   (activation Identity, per-partition bias)
  DVE    : acc  = min(acc, Qb)                     (TT bf16 SBUF, 2x mode)
           r1   = min(Qb[:, :1024], Qb[:, 1024:])  (TT 2x)
  DMA    : r1 streams out per tile ([128, 64, 1024] bf16 total)
acc ([128, 2048] bf16, min over m per (partition, n) position) is DMA'd out
raw; the rowmin tail (1024-way min per row), the cross-partition colmin, and
the Huber + sums happen on host (u16 bit-trick min, ~10 ms).
Tiny keypoint/KNN MSE losses run on-device at the end of every core.
"""

import os
import numpy as np


def _ensure_path():
    try:
        import concourse  # noqa: F401
    except ImportError:
        import sys
        for p in ("/opt/trn_rl_repo", "/root/.axon_site/_ro/trn_rl_repo"):
            if os.path.isdir(p) and p not in sys.path:
                sys.path.insert(0, p)


_ensure_path()

import ml_dtypes  # noqa: E402
import concourse.bass as bass  # noqa: E402
import concourse.bacc as bacc  # noqa: E402
import concourse.tile as tile  # noqa: E402
import concourse.mybir as mybir  # noqa: E402
from concourse.bass_utils import run_bass_kernel_spmd  # noqa: E402

F32 = mybir.dt.float32
BF16 = mybir.dt.bfloat16
AL = mybir.AluOpType
AF = mybir.ActivationFunctionType
BF16NP = np.dtype(ml_dtypes.bfloat16)

MARGIN = 0.1
B, KP, KNN, N, M = 2, 256, 32, 8192, 8192
NCORES = 8
NSHARDS = NCORES // B          # 4 n-chunks per batch element
CHUNK = N // NSHARDS           # 2048
NJ = CHUNK // 512              # 4 psum banks per m-tile
MI = M // 128                  # 64 m-tiles
K11 = 11
BIGB = 1.0e30

_CACHE = {}


def _build():
    nc = bacc.Bacc("TRN2", target_bir_lowering=False, debug=False,
                   num_devices=NCORES)

    tA_d = nc.dram_tensor("tA", [K11, M], BF16, kind="ExternalInput")
    T0 = 8 * 128                   # first 8 m-tiles land in tA0
    sA_d = nc.dram_tensor("sA", [K11, CHUNK], BF16, kind="ExternalInput")
    nt_d = nc.dram_tensor("nt", [128, MI], F32, kind="ExternalInput")
    kp_lhsT = nc.dram_tensor("kp_lhsT", [4, 2 * 3], F32, kind="ExternalInput")
    kp_rhs = nc.dram_tensor("kp_rhs", [4, 2 * KP], F32, kind="ExternalInput")
    tgt_kp = nc.dram_tensor("tgt_kp", [3, 2 * KP], F32, kind="ExternalInput")
    knn_src = nc.dram_tensor("knn_src", [128, 2 * 192], F32, kind="ExternalInput")
    knn_tgt = nc.dram_tensor("knn_tgt", [128, 2 * 192], F32, kind="ExternalInput")

    acc_o = nc.dram_tensor("acc", [128, CHUNK], BF16, kind="ExternalOutput")
    r1_o = nc.dram_tensor("r1o", [128, MI, CHUNK // 2], BF16,
                          kind="ExternalOutput")
    misc_o = nc.dram_tensor("misc", [128, 4], F32, kind="ExternalOutput")

    with tile.TileContext(nc) as tc:
        with (
            tc.tile_pool(name="const", bufs=1) as const,
            tc.tile_pool(name="qb", bufs=4) as qbp,
            tc.tile_pool(name="rt", bufs=4) as rtp,
        ):
            tA0 = const.tile([K11, T0], BF16)
            tA1 = const.tile([K11, M - T0], BF16)
            sA = const.tile([K11, CHUNK], BF16)
            nt_sb = const.tile([128, MI], F32)
            acc = const.tile([128, CHUNK], BF16)
            misc_sb = const.tile([128, 4], F32)
            dummy = const.tile([1, 1], F32)

            # preload the activation table off the critical path
            nc.scalar.add(out=dummy[:], in_=dummy[:], add=0.0)
            nc.sync.dma_start(out=sA[:], in_=sA_d[:])
            nc.sync.dma_start(out=tA0[:], in_=tA_d[:, :T0])
            nc.sync.dma_start(out=nt_sb[:], in_=nt_d[:])
            nc.sync.dma_start(out=tA1[:], in_=tA_d[:, T0:])
            nc.gpsimd.memset(acc[:], BIGB)
            nc.gpsimd.memset(misc_sb[:], 0.0)

            # tiny keypoint / knn losses first: fills the pipeline-fill bubble
            with tc.tile_pool(name="psum_fin", bufs=2, space="PSUM") as pf:
                kp_l = const.tile([4, 2 * 3], F32)
                kp_r = const.tile([4, 2 * KP], F32)
                kp_t = const.tile([3, 2 * KP], F32)
                ks = const.tile([128, 2 * 192], F32)
                kt = const.tile([128, 2 * 192], F32)
                nc.sync.dma_start(out=kp_l[:], in_=kp_lhsT[:])
                nc.sync.dma_start(out=kp_r[:], in_=kp_rhs[:])
                nc.sync.dma_start(out=kp_t[:], in_=tgt_kp[:])
                nc.sync.dma_start(out=ks[:], in_=knn_src[:])
                nc.sync.dma_start(out=kt[:], in_=knn_tgt[:])
                for b in range(B):
                    pt2 = pf.tile([3, KP], F32, tag="kp")
                    nc.tensor.matmul(
                        pt2[:], lhsT=kp_l[:, b * 3:(b + 1) * 3],
                        rhs=kp_r[:, b * KP:(b + 1) * KP],
                        start=True, stop=True)
                    diff = rtp.tile([3, KP], F32, tag="kdiff")
                    nc.vector.tensor_sub(diff[:], pt2[:],
                                         kp_t[:, b * KP:(b + 1) * KP])
                    nc.vector.tensor_mul(diff[:], diff[:], diff[:])
                    nc.vector.tensor_reduce(
                        out=misc_sb[0:3, b:b + 1], in_=diff[:],
                        axis=mybir.AxisListType.X, op=AL.add)
                    diff2 = rtp.tile([128, 192], F32, tag="ndiff")
                    nc.vector.tensor_sub(diff2[:], ks[:, b * 192:(b + 1) * 192],
                                         kt[:, b * 192:(b + 1) * 192])
                    nc.vector.tensor_mul(diff2[:], diff2[:], diff2[:])
                    nc.vector.tensor_reduce(
                        out=misc_sb[:, 2 + b:3 + b], in_=diff2[:],
                        axis=mybir.AxisListType.X, op=AL.add)
            nc.sync.dma_start(out=misc_o[:], in_=misc_sb[:])

            with tc.tile_pool(name="psum_main", bufs=2, space="PSUM") as pm:
                for mi in range(MI):
                    pt = pm.tile([128, CHUNK], F32, tag="pt")
                    if mi < 8:
                        lhsT = tA0[:, mi * 128:(mi + 1) * 128]
                    else:
                        lhsT = tA1[:, (mi - 8) * 128:(mi - 7) * 128]
                    for nj in range(NJ):
                        nc.tensor.matmul(
                            pt[:, nj * 512:(nj + 1) * 512],
                            lhsT=lhsT,
                            rhs=sA[:, nj * 512:(nj + 1) * 512],
                            start=True, stop=True,
                        )
                    qb = qbp.tile([128, CHUNK], BF16, tag="qb")
                    nc.scalar.add(out=qb[:], in_=pt[:], add=nt_sb[:, mi:mi + 1])
                    # colmin accumulate (min over m-tiles per n position)
                    nc.vector.tensor_tensor(acc[:], acc[:], qb[:], AL.min)
                    # rowmin tree (min over the n-chunk per m row)
                    r1 = rtp.tile([128, CHUNK // 2], BF16, tag="r1")
                    nc.vector.tensor_tensor(
                        r1[:], qb[:, :CHUNK // 2], qb[:, CHUNK // 2:], AL.min)
                    nc.sync.dma_start(out=r1_o[:, mi, :], in_=r1[:])

            nc.sync.dma_start(out=acc_o[:], in_=acc[:])

    nc.compile()
    return nc


def _get_nc():
    if "nc" not in _CACHE:
        _CACHE["nc"] = _build()
    return _CACHE["nc"]


def _split2(x):
    """Exact 2-way bf16 split of an fp32 array: x ~= h + m."""
    f = np.float32
    h = x.astype(BF16NP)
    m = (x - h.astype(f)).astype(BF16NP)
    return h, m


# lhsT rows [th,th,tm]*3d + ones*2 ; rhs rows [sh,sm,sh]*3d + 2-way split of
# ||s||^2.  (keeps the 3 dominant cross products; tm*sm term ~2^-18 dropped)
_T_DEST = {0: [0, 3], 1: [6]}    # th, tm row bases
_S_DEST = {0: [0, 6], 1: [3]}    # sh, sm row bases


def _pack_rows(x, ns, width):
    """Build the [11, width] bf16 operand for one side.

    x: [3, width] fp32 (already scaled by -2 for the t side)
    ns: [width] fp32 squared-norm rows (s side) or None (t side -> ones)
    """
    out = np.zeros((K11, width), dtype=BF16NP)
    h, m = _split2(x)
    dest = _S_DEST if ns is not None else _T_DEST
    for lvl, w in enumerate((h, m)):
        for base in dest[lvl]:
            out[base:base + 3, :] = w
    if ns is None:
        out[9:11, :] = np.ones((2, width), dtype=BF16NP)
    else:
        nh, nm = _split2(ns)
        out[9, :] = nh
        out[10, :] = nm
    return out


def _prepare_in_maps(src_keypoints, tgt_keypoints, rotation_ab, translation_ab,
                     src_keypoints_knn, tgt_keypoints_knn, src_transformed, tgt):
    f = np.float32
    st = np.ascontiguousarray(np.asarray(src_transformed, dtype=f))
    tg = np.ascontiguousarray(np.asarray(tgt, dtype=f))
    skp = np.asarray(src_keypoints, dtype=f)
    tkp = np.asarray(tgt_keypoints, dtype=f)
    rot = np.asarray(rotation_ab, dtype=f)
    tra = np.asarray(translation_ab, dtype=f)
    sknn = np.asarray(src_keypoints_knn, dtype=f)
    tknn = np.asarray(tgt_keypoints_knn, dtype=f)

    kp_lhsT = np.zeros((4, 2 * 3), dtype=f)
    kp_rhs = np.zeros((4, 2 * KP), dtype=f)
    tgt_kp = np.zeros((3, 2 * KP), dtype=f)
    knn_src = np.zeros((128, 2 * 192), dtype=f)
    knn_tgt = np.zeros((128, 2 * 192), dtype=f)
    for b in range(B):
        kp_lhsT[0:3, b * 3:(b + 1) * 3] = rot[b].T
        kp_lhsT[3, b * 3:(b + 1) * 3] = tra[b]
        kp_rhs[0:3, b * KP:(b + 1) * KP] = skp[b]
        kp_rhs[3, b * KP:(b + 1) * KP] = 1.0
        tgt_kp[:, b * KP:(b + 1) * KP] = tkp[b]
        knn_src[:, b * 192:(b + 1) * 192] = sknn[b].reshape(128, 192)
        knn_tgt[:, b * 192:(b + 1) * 192] = tknn[b].reshape(128, 192)

    shared = {
        "kp_lhsT": kp_lhsT, "kp_rhs": kp_rhs,
        "tgt_kp": tgt_kp, "knn_src": knn_src, "knn_tgt": knn_tgt,
    }
    # per-batch t-side operand + ||t||^2 in the [p, g] layout (m = g*128 + p)
    tA_b, nt_b = [], []
    for b in range(B):
        t = tg[b]                                   # [3, M]
        tA_b.append(_pack_rows(-2.0 * t, None, M))
        nt = (t * t).sum(axis=0)                    # [M]
        nt_b.append(np.ascontiguousarray(nt.reshape(MI, 128).T))
    in_maps = []
    for ci in range(NCORES):
        b, j = divmod(ci, NSHARDS)
        s = np.ascontiguousarray(st[b, :, j * CHUNK:(j + 1) * CHUNK])
        ns = (s * s).sum(axis=0)
        mdict = dict(shared)
        mdict["tA"] = tA_b[b]
        mdict["nt"] = nt_b[b]
        mdict["sA"] = _pack_rows(s, ns, CHUNK)
        in_maps.append(mdict)
    return in_maps


def _huber(x, c):
    return np.where(x < c, 0.5 * x * x, c * x - 0.5 * c * c)


def _rowmin_host(r1o):
    """Per-row min over the last axis of a [128, MI, CHUNK//2] bf16 array.

    Uses the uint16 bit-pattern trick (valid for non-negative bf16); falls
    back to fp32 if any negative value is present (only possible within
    ~1e-4 of zero, where Huber is ~0 either way).
    """
    u = np.asarray(r1o).view(np.uint16)
    if (u & 0x8000).any():
        return np.asarray(r1o).astype(np.float32).min(axis=-1)
    return u.min(axis=-1).view(BF16NP).astype(np.float32)


def _postprocess(results):
    c = np.float64(MARGIN)
    loss1 = np.float64(0.0)
    loss2 = np.float64(0.0)
    for b in range(B):
        rowmins = []
        for j in range(NSHARDS):
            r = results[b * NSHARDS + j]
            colmin = np.asarray(r["acc"]).astype(np.float64).min(axis=0)
            loss1 += _huber(colmin, c).sum()
            rm_core = _rowmin_host(r["r1o"])          # [128, MI]
            rowmins.append(rm_core.astype(np.float64).T.ravel())
        rm = np.minimum.reduce(rowmins)
        loss2 += _huber(rm, c).sum()
    gal = loss1 + loss2

    misc = np.asarray(results[0]["misc"], dtype=np.float64)
    kp_loss = (misc[0:3, 0].sum() + misc[0:3, 1].sum()) / B
    knn_loss = (misc[:, 2].sum() + misc[:, 3].sum()) / (B * KNN)
    ncl = knn_loss + kp_loss
    return np.float32(ncl), np.float32(gal)


def run_device(in_maps, **kw):
    nc = _get_nc()
    return run_bass_kernel_spmd(nc, in_maps, list(range(NCORES)), **kw)


def kernel(src_keypoints, tgt_keypoints, rotation_ab, translation_ab,
           src_keypoints_knn, tgt_keypoints_knn, k, src_transformed, tgt,
           **_unused):
    in_maps = _prepare_in_maps(src_keypoints, tgt_keypoints, rotation_ab,
                               translation_ab, src_keypoints_knn,
                               tgt_keypoints_knn, src_transformed, tgt)
    res = run_device(in_maps)
    return _postprocess(res.results)


# revision 13
# speedup vs baseline: 1.0395x; 1.0395x over previous
"""RIENet loss kernel (keypoint/KNN MSE + global-align Huber-min loss) on 8 trn2 cores.

Sharding: core ci -> (b = ci // 4, n-chunk j = ci % 4).  Each core holds the full
tgt[b] (M=8192 points) and a 2048-column chunk of src_transformed[b] (N axis).
  loss_1 (min over M per src point): per-core partial over the partition axis,
          finished on host (min over 128 partitions of the DMA'd acc tile).
  loss_2 (min over N per tgt point): per-core partial min over its chunk;
          host min-reduces the 4 chunks per batch element.

v5: all operand prep happens on the HOST (bf16 2-way splits, norms, row
layouts) so the device runs only the steady-state loop:
  PE     : Q[m-tile, :] = -2 t.s + ||s||^2 via K=11 bf16 matmuls (4 banks)
  ScalarE: Qb = bf16(Q + ||t_m||^2)   (activation Identity, per-partition bias)
  DVE    : acc  = min(acc, Qb)                     (TT bf16 SBUF, 2x mode)
           r1   = min(Qb[:, :1024], Qb[:, 1024:])  (TT 2x)
  DMA    : r1 streams out per tile ([128, 64, 1024] bf16 total)
acc ([128, 2048] bf16, min over m per (partition, n) position) is DMA'd out
raw; the rowmin tail (1024-way min per row), the cross-partition colmin, and
the Huber + sums happen on host (u16 bit-trick min, ~10 ms).
Tiny keypoint/KNN MSE losses run on-device at the end of every core.
"""

import os
import numpy as np


def _ensure_path():
    try:
        import concourse  # noqa: F401
    except ImportError:
        import sys
        for p in ("/opt/trn_rl_repo", "/root/.axon_site/_ro/trn_rl_repo"):
            if os.path.isdir(p) and p not in sys.path:
                sys.path.insert(0, p)


_ensure_path()

import ml_dtypes  # noqa: E402
import concourse.bass as bass  # noqa: E402
import concourse.bacc as bacc  # noqa: E402
import concourse.tile as tile  # noqa: E402
import concourse.mybir as mybir  # noqa: E402
from concourse.bass_utils import run_bass_kernel_spmd  # noqa: E402

F32 = mybir.dt.float32
BF16 = mybir.dt.bfloat16
AL = mybir.AluOpType
AF = mybir.ActivationFunctionType
BF16NP = np.dtype(ml_dtypes.bfloat16)

MARGIN = 0.1
B, KP, KNN, N, M = 2, 256, 32, 8192, 8192
NCORES = 8
NSHARDS = NCORES // B          # 4 n-chunks per batch element
CHUNK = N // NSHARDS           # 2048
NJ = CHUNK // 512              # 4 psum banks per m-tile
MI = M // 128                  # 64 m-tiles
K11 = 11
BIGB = 1.0e30

_CACHE = {}


def _build():
    nc = bacc.Bacc("TRN2", target_bir_lowering=False, debug=False,
                   num_devices=NCORES)

    tA_d = nc.dram_tensor("tA", [K11, M], BF16, kind="ExternalInput")
    sA_d = nc.dram_tensor("sA", [K11, CHUNK], BF16, kind="ExternalInput")
    nt_d = nc.dram_tensor("nt", [128, MI], F32, kind="ExternalInput")
    kp_lhsT = nc.dram_tensor("kp_lhsT", [4, 2 * 3], F32, kind="ExternalInput")
    kp_rhs = nc.dram_tensor("kp_rhs", [4, 2 * KP], F32, kind="ExternalInput")
    tgt_kp = nc.dram_tensor("tgt_kp", [3, 2 * KP], F32, kind="ExternalInput")
    knn_src = nc.dram_tensor("knn_src", [128, 2 * 192], F32, kind="ExternalInput")
    knn_tgt = nc.dram_tensor("knn_tgt", [128, 2 * 192], F32, kind="ExternalInput")

    acc_o = nc.dram_tensor("acc", [128, CHUNK], BF16, kind="ExternalOutput")
    r1_o = nc.dram_tensor("r1o", [128, MI, CHUNK // 2], BF16,
                          kind="ExternalOutput")
    misc_o = nc.dram_tensor("misc", [128, 4], F32, kind="ExternalOutput")

    with tile.TileContext(nc) as tc:
        with (
            tc.tile_pool(name="const", bufs=1) as const,
            tc.tile_pool(name="qb", bufs=3) as qbp,
            tc.tile_pool(name="rt", bufs=3) as rtp,
        ):
            tA = const.tile([K11, M], BF16)
            sA = const.tile([K11, CHUNK], BF16)
            nt_sb = const.tile([128, MI], F32)
            acc = const.tile([128, CHUNK], BF16)
            misc_sb = const.tile([128, 4], F32)
            dummy = const.tile([1, 1], F32)

            # preload the activation table off the critical path
            nc.scalar.add(out=dummy[:], in_=dummy[:], add=0.0)
            nc.sync.dma_start(out=sA[:], in_=sA_d[:])
            nc.sync.dma_start(out=tA[:], in_=tA_d[:])
            nc.sync.dma_start(out=nt_sb[:], in_=nt_d[:])
            nc.gpsimd.memset(acc[:], BIGB)
            nc.gpsimd.memset(misc_sb[:], 0.0)


            with tc.tile_pool(name="psum_main", bufs=2, space="PSUM") as pm:
                for mi in range(MI):
                    pt = pm.tile([128, CHUNK], F32, tag="pt")
                    for nj in range(NJ):
                        nc.tensor.matmul(
                            pt[:, nj * 512:(nj + 1) * 512],
                            lhsT=tA[:, mi * 128:(mi + 1) * 128],
                            rhs=sA[:, nj * 512:(nj + 1) * 512],
                            start=True, stop=True,
                        )
                    qb = qbp.tile([128, CHUNK], BF16, tag="qb")
                    nc.scalar.add(out=qb[:], in_=pt[:], add=nt_sb[:, mi:mi + 1])
                    # colmin accumulate (min over m-tiles per n position)
                    nc.vector.tensor_tensor(acc[:], acc[:], qb[:], AL.min)
                    # rowmin tree (min over the n-chunk per m row)
                    r1 = rtp.tile([128, CHUNK // 2], BF16, tag="r1")
                    nc.vector.tensor_tensor(
                        r1[:], qb[:, :CHUNK // 2], qb[:, CHUNK // 2:], AL.min)
                    nc.sync.dma_start(out=r1_o[:, mi, :], in_=r1[:])

            nc.sync.dma_start(out=acc_o[:], in_=acc[:])

            # tiny keypoint / knn losses (both batch elements)
            with tc.tile_pool(name="psum_fin", bufs=2, space="PSUM") as pf:
                kp_l = const.tile([4, 2 * 3], F32)
                kp_r = const.tile([4, 2 * KP], F32)
                kp_t = const.tile([3, 2 * KP], F32)
                ks = const.tile([128, 2 * 192], F32)
                kt = const.tile([128, 2 * 192], F32)
                nc.sync.dma_start(out=kp_l[:], in_=kp_lhsT[:])
                nc.sync.dma_start(out=kp_r[:], in_=kp_rhs[:])
                nc.sync.dma_start(out=kp_t[:], in_=tgt_kp[:])
                nc.sync.dma_start(out=ks[:], in_=knn_src[:])
                nc.sync.dma_start(out=kt[:], in_=knn_tgt[:])
                for b in range(B):
                    pt2 = pf.tile([3, KP], F32, tag="kp")
                    nc.tensor.matmul(
                        pt2[:], lhsT=kp_l[:, b * 3:(b + 1) * 3],
                        rhs=kp_r[:, b * KP:(b + 1) * KP],
                        start=True, stop=True)
                    diff = rtp.tile([3, KP], F32, tag="kdiff")
                    nc.vector.tensor_sub(diff[:], pt2[:],
                                         kp_t[:, b * KP:(b + 1) * KP])
                    nc.vector.tensor_mul(diff[:], diff[:], diff[:])
                    nc.vector.tensor_reduce(
                        out=misc_sb[0:3, b:b + 1], in_=diff[:],
                        axis=mybir.AxisListType.X, op=AL.add)
                    diff2 = rtp.tile([128, 192], F32, tag="ndiff")
                    nc.vector.tensor_sub(diff2[:], ks[:, b * 192:(b + 1) * 192],
                                         kt[:, b * 192:(b + 1) * 192])
                    nc.vector.tensor_mul(diff2[:], diff2[:], diff2[:])
                    nc.vector.tensor_reduce(
                        out=misc_sb[:, 2 + b:3 + b], in_=diff2[:],
                        axis=mybir.AxisListType.X, op=AL.add)

            nc.sync.dma_start(out=misc_o[:], in_=misc_sb[:])

    nc.compile()
    return nc


def _get_nc():
    if "nc" not in _CACHE:
        _CACHE["nc"] = _build()
    return _CACHE["nc"]


def _split2(x):
    """Exact 2-way bf16 split of an fp32 array: x ~= h + m."""
    f = np.float32
    h = x.astype(BF16NP)
    m = (x - h.astype(f)).astype(BF16NP)
    return h, m


# lhsT rows [th,th,tm]*3d + ones*2 ; rhs rows [sh,sm,sh]*3d + 2-way split of
# ||s||^2.  (keeps the 3 dominant cross products; tm*sm term ~2^-18 dropped)
_T_DEST = {0: [0, 3], 1: [6]}    # th, tm row bases
_S_DEST = {0: [0, 6], 1: [3]}    # sh, sm row bases


def _pack_rows(x, ns, width):
    """Build the [11, width] bf16 operand for one side.

    x: [3, width] fp32 (already scaled by -2 for the t side)
    ns: [width] fp32 squared-norm rows (s side) or None (t side -> ones)
    """
    out = np.zeros((K11, width), dtype=BF16NP)
    h, m = _split2(x)
    dest = _S_DEST if ns is not None else _T_DEST
    for lvl, w in enumerate((h, m)):
        for base in dest[lvl]:
            out[base:base + 3, :] = w
    if ns is None:
        out[9:11, :] = np.ones((2, width), dtype=BF16NP)
    else:
        nh, nm = _split2(ns)
        out[9, :] = nh
        out[10, :] = nm
    return out


def _prepare_in_maps(src_keypoints, tgt_keypoints, rotation_ab, translation_ab,
                     src_keypoints_knn, tgt_keypoints_knn, src_transformed, tgt):
    f = np.float32
    st = np.ascontiguousarray(np.asarray(src_transformed, dtype=f))
    tg = np.ascontiguousarray(np.asarray(tgt, dtype=f))
    skp = np.asarray(src_keypoints, dtype=f)
    tkp = np.asarray(tgt_keypoints, dtype=f)
    rot = np.asarray(rotation_ab, dtype=f)
    tra = np.asarray(translation_ab, dtype=f)
    sknn = np.asarray(src_keypoints_knn, dtype=f)
    tknn = np.asarray(tgt_keypoints_knn, dtype=f)

    kp_lhsT = np.zeros((4, 2 * 3), dtype=f)
    kp_rhs = np.zeros((4, 2 * KP), dtype=f)
    tgt_kp = np.zeros((3, 2 * KP), dtype=f)
    knn_src = np.zeros((128, 2 * 192), dtype=f)
    knn_tgt = np.zeros((128, 2 * 192), dtype=f)
    for b in range(B):
        kp_lhsT[0:3, b * 3:(b + 1) * 3] = rot[b].T
        kp_lhsT[3, b * 3:(b + 1) * 3] = tra[b]
        kp_rhs[0:3, b * KP:(b + 1) * KP] = skp[b]
        kp_rhs[3, b * KP:(b + 1) * KP] = 1.0
        tgt_kp[:, b * KP:(b + 1) * KP] = tkp[b]
        knn_src[:, b * 192:(b + 1) * 192] = sknn[b].reshape(128, 192)
        knn_tgt[:, b * 192:(b + 1) * 192] = tknn[b].reshape(128, 192)

    shared = {
        "kp_lhsT": kp_lhsT, "kp_rhs": kp_rhs,
        "tgt_kp": tgt_kp, "knn_src": knn_src, "knn_tgt": knn_tgt,
    }
    # per-batch t-side operand + ||t||^2 in the [p, g] layout (m = g*128 + p)
    tA_b, nt_b = [], []
    for b in range(B):
        t = tg[b]                                   # [3, M]
        tA_b.append(_pack_rows(-2.0 * t, None, M))
        nt = (t * t).sum(axis=0)                    # [M]
        nt_b.append(np.ascontiguousarray(nt.reshape(MI, 128).T))
    in_maps = []
    for ci in range(NCORES):
        b, j = divmod(ci, NSHARDS)
        s = np.ascontiguousarray(st[b, :, j * CHUNK:(j + 1) * CHUNK])
        ns = (s * s).sum(axis=0)
        mdict = dict(shared)
        mdict["tA"] = tA_b[b]
        mdict["nt"] = nt_b[b]
        mdict["sA"] = _pack_rows(s, ns, CHUNK)
        in_maps.append(mdict)
    return in_maps


def _huber(x, c):
    return np.where(x < c, 0.5 * x * x, c * x - 0.5 * c * c)


def _rowmin_host(r1o):
    """Per-row min over the last axis of a [128, MI, CHUNK//2] bf16 array.

    Uses the uint16 bit-pattern trick (valid for non-negative bf16); falls
    back to fp32 if any negative value is present (only possible within
    ~1e-4 of zero, where Huber is ~0 either way).
    """
    u = np.asarray(r1o).view(np.uint16)
    if (u & 0x8000).any():
        return np.asarray(r1o).astype(np.float32).min(axis=-1)
    return u.min(axis=-1).view(BF16NP).astype(np.float32)


def _postprocess(results):
    c = np.float64(MARGIN)
    loss1 = np.float64(0.0)
    loss2 = np.float64(0.0)
    for b in range(B):
        rowmins = []
        for j in range(NSHARDS):
            r = results[b * NSHARDS + j]
            colmin = np.asarray(r["acc"]).astype(np.float64).min(axis=0)
            loss1 += _huber(colmin, c).sum()
            rm_core = _rowmin_host(r["r1o"])          # [128, MI]
            rowmins.append(rm_core.astype(np.float64).T.ravel())
        rm = np.minimum.reduce(rowmins)
        loss2 += _huber(rm, c).sum()
    gal = loss1 + loss2

    misc = np.asarray(results[0]["misc"], dtype=np.float64)
    kp_loss = (misc[0:3, 0].sum() + misc[0:3, 1].sum()) / B
    knn_loss = (misc[:, 2].sum() + misc[:, 3].sum()) / (B * KNN)
    ncl = knn_loss + kp_loss
    return np.float32(ncl), np.float32(gal)


def run_device(in_maps, **kw):
    nc = _get_nc()
    return run_bass_kernel_spmd(nc, in_maps, list(range(NCORES)), **kw)


def kernel(src_keypoints, tgt_keypoints, rotation_ab, translation_ab,
           src_keypoints_knn, tgt_keypoints_knn, k, src_transformed, tgt,
           **_unused):
    in_maps = _prepare_in_maps(src_keypoints, tgt_keypoints, rotation_ab,
                               translation_ab, src_keypoints_knn,
                               tgt_keypoints_knn, src_transformed, tgt)
    res = run_device(in_maps)
    return _postprocess(res.results)
